# revision 18
# baseline (speedup 1.0000x reference)
"""Trainium2 Bass kernel for nn_DataEmbedding_v2 — v3.

Same contract as before. v3 replaces the O(S^2) DVE compare/max tc-plane with a
two-level block algorithm:
  - norms per 128-token block -> block minima (exact f32 via PE transpose)
  - level 1: for each t, last block j* with blockmin_j < thresh[t] (tiny DVE)
  - level 2: gather block j*'s norms via a one-hot PE matmul that also forms
    d = thresh[t] - norms[s] exactly (f16 hi/mid/lo 3-way splits, interleaved
    accumulation order so all partial sums stay cancellation-exact)
  - diag block (s in t's own block) via the same d-matmul with static rhs rows
  - ACT Sign(d) -> +-1, DVE multiply by (s+1) value masks, max-reduce; negative
    candidates lose automatically, "none" stays <= 0.
All compares bit-match the reference's f32 rounding.
"""

import math
import os
import sys

sys.path.insert(0, "/opt/trn_rl_repo")

import numpy as np

B, S, C, D = 16, 2048, 64, 512
NCORES = 8
BLOC = B // NCORES
P = 128
NT = S // P
ETA = 0.3
EPS = 1e-08
OG = 4  # output tiles per DMA group
BIG = 60000.0


def _emit(tc, aps):
    from contextlib import ExitStack

    from concourse import mybir

    f32 = mybir.dt.float32
    f16 = mybir.dt.float16
    Alu = mybir.AluOpType
    Ax = mybir.AxisListType
    Af = mybir.ActivationFunctionType

    nc = tc.nc
    xin, pe16, wt01, wt2 = aps["xin"], aps["pe16"], aps["wt01"], aps["wt2"]
    tcwhl, identh, identf = aps["tcwhl"], aps["identh"], aps["identf"]
    onesrow, mask1, kcol = aps["onesrow"], aps["mask1"], aps["kcol"]
    dval, sval, t1c = aps["dval"], aps["sval"], aps["t1c"]
    out, xta = aps["out"], aps["xta"]
    lhsTd_t, rhsd_t, nbt_t = aps["lhsTd_t"], aps["rhsd_t"], aps["nbt_t"]
    js_dram, tau_dram = aps["js_dram"], aps["tau_dram"]
    spT_d = aps["spT_d"]

    with ExitStack() as ctx:
        singles = ctx.enter_context(tc.tile_pool(name="singles", bufs=1))
        xpool = ctx.enter_context(tc.tile_pool(name="xpool", bufs=2))
        small = ctx.enter_context(tc.tile_pool(name="small", bufs=2))
        selp = ctx.enter_context(tc.tile_pool(name="selp", bufs=2))
        xtp = ctx.enter_context(tc.tile_pool(name="xtp", bufs=2))
        sgp = ctx.enter_context(tc.tile_pool(name="sgp", bufs=2))
        outp = ctx.enter_context(tc.tile_pool(name="outp", bufs=3))
        psA = ctx.enter_context(tc.tile_pool(name="psA", bufs=5, space="PSUM"))
        psT = ctx.enter_context(tc.tile_pool(name="psT", bufs=1, space="PSUM"))
        psD = ctx.enter_context(tc.tile_pool(name="psD", bufs=1, space="PSUM"))

        # ---- x loads (scalar/gpsimd rings; sync ring starts on consts) ----
        xins = {}
        xtas = {}
        for b in range(BLOC):
            xin_sb = xpool.tile([P, NT, C], f32, tag="xin", name=f"xin_sb{b}")
            nc.scalar.dma_start(xin_sb, xin[b].rearrange("(j p) c -> p j c", p=P))
            xins[b] = xin_sb
            xta_sb = xtp.tile([P, S + 2], f16, tag="xta", name=f"xta{b}")
            nc.gpsimd.dma_start(xta_sb, xta[b])
            xtas[b] = xta_sb

        # ---- constants ----
        identh_sb = singles.tile([P, P], f16)
        nc.sync.dma_start(identh_sb, identh)
        identf_sb = singles.tile([P, P], f32)
        nc.sync.dma_start(identf_sb, identf)
        onesrow_sb = singles.tile([1, P], f32)
        nc.sync.dma_start(onesrow_sb, onesrow)
        mask1_sb = singles.tile([P, NT, NT], f16)
        nc.sync.dma_start(mask1_sb, mask1)
        kcol_sb = singles.tile([32, 1], f32)
        nc.sync.dma_start(kcol_sb, kcol)
        dval_sb = singles.tile([P, NT, P], f16)
        nc.sync.dma_start(dval_sb, dval)
        sval_sb = singles.tile([P, P], f16)
        nc.sync.dma_start(sval_sb, sval)
        t1_sb = singles.tile([P, NT], f32)
        nc.sync.dma_start(t1_sb, t1c)
        wt01_sb = singles.tile([P, D], f16)
        nc.sync.dma_start(wt01_sb, wt01)
        wt2_sb = singles.tile([P, D], f16)
        nc.sync.dma_start(wt2_sb[C : 2 * C, :], wt2)
        tcwhl_sb = singles.tile([34, D], f16)
        nc.sync.dma_start(tcwhl_sb[32:34, :], tcwhl[0])
        pe16_sb = singles.tile([P, NT, D], f16)
        nc.sync.dma_start(pe16_sb, pe16.rearrange("(i p) d -> p i d", p=P))

        # ---- PE priming: absorb each const-DMA dependency once ----
        prime_h = psA.tile([P, P], f16, tag="psa")
        nc.tensor.transpose(prime_h, identh_sb, identh_sb)
        prime_f = psA.tile([P, P], f32, tag="psa")
        nc.tensor.transpose(prime_f, identf_sb, identf_sb)
        prime_w2 = psA.tile([P, D], f32, tag="psa")
        nc.tensor.matmul(
            prime_w2[C : 2 * C, :],
            lhsT=wt2_sb[C : 2 * C, 0:C],
            rhs=wt2_sb[C : 2 * C, :],
            start=True,
            stop=True,
        )
        prime_tc = psA.tile([P, D], f32, tag="psa")
        nc.tensor.matmul(
            prime_tc,
            lhsT=tcwhl_sb[32:34, 0:P],
            rhs=tcwhl_sb[32:34, :],
            start=True,
            stop=True,
            tile_position=(32, 0),
        )
        prime_w01 = psA.tile([P, D], f32, tag="psa")
        nc.tensor.matmul(
            prime_w01, lhsT=wt01_sb[:, 0:P], rhs=wt01_sb, start=True, stop=True
        )
        prime_on = psA.tile([P, 16], f32, tag="psa")
        nc.tensor.matmul(
            prime_on, lhsT=onesrow_sb, rhs=onesrow_sb[:, 0:16], start=True, stop=True
        )
        dumps = singles.tile([P, 7], f32)
        nc.scalar.copy(dumps[:, 0:1], prime_h[:, 0:1])
        nc.scalar.copy(dumps[:, 1:2], prime_f[:, 0:1])
        nc.scalar.copy(dumps[C : 2 * C, 2:3], prime_w2[C : 2 * C, 0:1])
        nc.scalar.copy(dumps[:, 3:4], prime_tc[:, 0:1])
        nc.scalar.copy(dumps[:, 4:5], prime_w01[:, 0:1])
        nc.scalar.copy(dumps[:, 6:7], prime_on[:, 0:1])

        # ---- Phase A per batch: norms, splits, block minima, level 1 ----
        st = {}
        for b in range(BLOC):
            xin_sb = xins[b]
            r8 = small.tile([P, NT, 8], f32, tag="r8", name=f"r8_{b}")
            nc.vector.tensor_reduce(
                r8,
                xin_sb.rearrange("p j (a b) -> p j a b", b=8),
                axis=Ax.X,
                op=Alu.add,
                apply_absolute_value=True,
            )
            normc = small.tile([P, NT], f32, tag="normc", name=f"normc{b}")
            nc.vector.tensor_reduce(normc, r8, axis=Ax.X, op=Alu.add)
            # negthc[:,0,:] = -norms, negthc[:,1,:] = thresh = (norms+EPS)*0.7
            negthc = small.tile([P, 2, NT], f32, tag="negthc", name=f"negthc{b}")
            nc.vector.tensor_scalar(negthc[:, 0, :], normc, -1.0, None, op0=Alu.mult)
            nc.vector.tensor_scalar(
                negthc[:, 1, :],
                normc,
                float(EPS),
                float(1.0 - ETA),
                op0=Alu.add,
                op1=Alu.mult,
            )
            # f16 3-way splits of (-norms, thresh): exact hi/mid/lo
            # layout [P, {n,q}, k, NT] so the DMA slices below are contiguous
            spl = small.tile([P, 2, 3, NT], f16, tag="spl", name=f"spl{b}")
            nc.vector.tensor_copy(spl[:, :, 0, :], negthc)
            rem1 = small.tile([P, 2, NT], f32, tag="rem1", name=f"rem1_{b}")
            nc.vector.tensor_tensor(rem1, negthc, spl[:, :, 0, :], op=Alu.subtract)
            nc.vector.tensor_copy(spl[:, :, 1, :], rem1)
            rem2 = small.tile([P, 2, NT], f32, tag="rem2", name=f"rem2_{b}")
            nc.vector.tensor_tensor(rem2, rem1, spl[:, :, 1, :], op=Alu.subtract)
            nc.vector.tensor_copy(spl[:, :, 2, :], rem2)
            # transpose all six split vectors at once: spT[(nq)*48 + k*16 + i, m]
            # = split[(nq), k][t = i*128 + m]  (rows 0:48 = -n, 48:96 = q)
            ps96 = psT.tile([96, P], f16, tag="pst", name=f"ps96_{b}")
            nc.tensor.transpose(ps96, spl.rearrange("p a k j -> p (a k j)"), identh_sb)
            spT = small.tile([96, P], f16, tag="spt", name=f"spT{b}")
            nc.scalar.copy(spT, ps96)
            nc.scalar.dma_start(spT_d[b], spT)
            # block minima of norms  (= -max of -norms), exact f32
            psnT = psT.tile([NT, P], f32, tag="pst", name=f"psnT{b}")
            nc.tensor.transpose(psnT, negthc[:, 0, :], identf_sb)
            bneg = small.tile([NT, 1], f32, tag="bneg", name=f"bneg{b}")
            nc.vector.tensor_reduce(bneg, psnT, axis=Ax.X, op=Alu.max)
            psb1 = psT.tile([1, NT], f32, tag="pst", name=f"psb1_{b}")
            nc.tensor.transpose(psb1, bneg, identf_sb[0:NT, 0:NT])
            bnegrow = small.tile([1, NT], f32, tag="bnegrow", name=f"bnegrow{b}")
            nc.scalar.copy(bnegrow, psb1)
            psbb = psT.tile([P, NT], f32, tag="pst", name=f"psbb{b}")
            nc.tensor.matmul(
                psbb, lhsT=onesrow_sb, rhs=bnegrow, start=True, stop=True
            )
            bnegbc = small.tile([P, NT], f32, tag="bnegbc", name=f"bnegbc{b}")
            nc.scalar.copy(bnegbc, psbb)
            # level 1: jp1 = 1 + last block j<i with bmin_j < thresh  (0 = none)
            l1a = small.tile([P, NT, NT], f32, tag="l1a", name=f"l1a{b}")
            nc.vector.tensor_tensor(
                l1a,
                negthc[:, 1, :].unsqueeze(2).to_broadcast([P, NT, NT]),
                bnegbc.unsqueeze(1).to_broadcast([P, NT, NT]),
                op=Alu.add,
            )
            l1b = small.tile([P, NT, NT], f16, tag="l1b", name=f"l1b{b}")
            nc.vector.tensor_scalar(l1b, l1a, 0.0, None, op0=Alu.is_gt)
            l1c = small.tile([P, NT, NT], f16, tag="l1c", name=f"l1c{b}")
            nc.vector.tensor_tensor(l1c, l1b, mask1_sb, op=Alu.mult)
            jp1 = small.tile([P, NT], f16, tag="jp1", name=f"jp1_{b}")
            nc.vector.tensor_reduce(jp1, l1c, axis=Ax.X, op=Alu.max)
            psj = psT.tile([NT, P], f16, tag="pst", name=f"psj{b}")
            nc.tensor.transpose(psj, jp1, identh_sb)
            jpT = small.tile([NT, P], f16, tag="jpt", name=f"jpT{b}")
            nc.scalar.copy(jpT, psj)
            nc.sync.dma_start(js_dram[b].rearrange("(i p) -> i p", p=P), jpT)
            st[b] = (negthc, spl, jp1, spT)

        # ---- Phase B per batch: select matmuls, sign, reduce, tau ----
        st2 = {}
        for b in range(BLOC):
            negthc, spl, jp1, spT = st[b]
            xta_sb = xtas[b]
            # one-hot lhsT [96, S], 32-aligned groups (hi/mid/lo):
            # rows g*32+0..16 = one-hot (17), g*32+17 = q_g, rest zeroed
            ohx = selp.tile([96, S], f16, tag="ohx", name=f"ohx{b}")
            ohsrc = selp.tile([32, S], f16, tag="ohsrc", name=f"ohsrc{b}")
            nc.gpsimd.dma_start(ohsrc, js_dram[b].partition_broadcast(32))
            nc.vector.tensor_scalar(
                ohx[0:32, :], ohsrc, kcol_sb, None, op0=Alu.is_equal
            )
            nc.vector.tensor_scalar(
                ohx[32:64, :], ohsrc, kcol_sb, None, op0=Alu.is_equal
            )
            nc.vector.tensor_scalar(
                ohx[64:96, :], ohsrc, kcol_sb, None, op0=Alu.is_equal
            )
            nc.scalar.dma_start(
                ohx[17:82:32, :], spT_d[b, 48:96].rearrange("(k i) m -> k (i m)", k=3)
            )
            # diag lhsT [6, S]: ones rows 0/2/4, q rows 1/3/5
            lhsTd = selp.tile([6, S], f16, tag="lhstd", name=f"lhsTd{b}")
            nc.gpsimd.dma_start(lhsTd, lhsTd_t)
            nc.scalar.dma_start(
                lhsTd[1:6:2, :], spT_d[b, 48:96].rearrange("(k i) m -> k (i m)", k=3)
            )
            # diag rhs [6, S]: -n rows 0/2/4, ones rows 1/3/5
            rhsd = selp.tile([6, S], f16, tag="rhsd", name=f"rhsd{b}")
            nc.gpsimd.dma_start(rhsd, rhsd_t)
            nc.scalar.dma_start(
                rhsd[0:6:2, :], spT_d[b, 0:48].rearrange("(k i) m -> k (i m)", k=3)
            )
            # one-hot rhs tables [54, P]: -n blocks hi/mid/lo + specials
            nbt = selp.tile([96, P], f16, tag="nbt", name=f"nbt{b}")
            nc.gpsimd.dma_start(nbt, nbt_t)
            for g in range(3):
                nc.sync.dma_start(
                    nbt[g * 32 : g * 32 + 16, :], spT_d[b, g * 16 : (g + 1) * 16]
                )

            # absorb multi-writer waits before the select matmuls
            nc.tensor.ldweights(lhsTd[:, 0:P])
            nc.tensor.ldweights(rhsd[:, 0:P])
            nc.tensor.ldweights(ohx[:, 0:P])
            nc.tensor.ldweights(nbt[:, 0:P])

            rdiag = small.tile([P, NT], f16, tag="rdiag", name=f"rdiag{b}")
            rsel = small.tile([P, NT], f16, tag="rsel", name=f"rsel{b}")
            for h in range(4):
                psd = psD.tile([P, 4, 2, P], f32, tag="psd", name=f"psd{b}_{h}")
                for ii in range(4):
                    i = h * 4 + ii
                    sl = slice(i * P, (i + 1) * P)
                    nc.tensor.matmul(
                        psd[:, ii, 0, :],
                        lhsT=lhsTd[:, sl],
                        rhs=rhsd[:, sl],
                        start=True,
                        stop=True,
                    )
                    nc.tensor.matmul(
                        psd[:, ii, 1, :],
                        lhsT=ohx[:, sl],
                        rhs=nbt,
                        start=True,
                        stop=True,
                    )
                sgn = sgp.tile([P, 4, 2, P], f16, tag="sgn", name=f"sgn{b}_{h}")
                nc.scalar.activation(sgn, psd, Af.Sign)
                dm = sgp.tile([P, 4, P], f16, tag="dm", name=f"dm{b}_{h}")
                nc.vector.tensor_tensor(
                    dm, sgn[:, :, 0, :], dval_sb[:, h * 4 : (h + 1) * 4, :],
                    op=Alu.mult,
                )
                sm = sgp.tile([P, 4, P], f16, tag="sm", name=f"sm{b}_{h}")
                nc.vector.tensor_tensor(
                    sm,
                    sgn[:, :, 1, :],
                    sval_sb.unsqueeze(1).to_broadcast([P, 4, P]),
                    op=Alu.mult,
                )
                nc.vector.tensor_reduce(
                    rdiag[:, h * 4 : (h + 1) * 4], dm, axis=Ax.X, op=Alu.max
                )
                nc.vector.tensor_reduce(
                    rsel[:, h * 4 : (h + 1) * 4], sm, axis=Ax.X, op=Alu.max
                )

            # ans = max(diag, (jp1-1)*128 + rsel);  tau = ans>0 ? t+1-ans : 0
            cand2 = small.tile([P, NT], f16, tag="cand2", name=f"cand2_{b}")
            nc.vector.tensor_scalar(
                cand2, jp1, 128.0, -128.0, op0=Alu.mult, op1=Alu.add
            )
            cand2b = small.tile([P, NT], f16, tag="cand2b", name=f"cand2b{b}")
            nc.vector.tensor_tensor(cand2b, cand2, rsel, op=Alu.add)
            ans = small.tile([P, NT], f16, tag="ans", name=f"ans{b}")
            nc.vector.tensor_tensor(ans, cand2b, rdiag, op=Alu.max)
            m01 = small.tile([P, NT], f16, tag="m01", name=f"m01_{b}")
            nc.vector.tensor_scalar(m01, ans, 0.0, None, op0=Alu.is_gt)
            td = small.tile([P, NT], f32, tag="td", name=f"td{b}")
            nc.vector.tensor_tensor(td, t1_sb, ans, op=Alu.subtract)
            tauc = small.tile([P, NT], f32, tag="tauc", name=f"tauc{b}")
            nc.vector.tensor_tensor(tauc, td, m01, op=Alu.mult)
            nc.sync.dma_start(tau_dram[b].rearrange("(p j) -> p j", p=P), tauc)
            taurow = small.tile([34, S], f16, tag="taurow", name=f"taurow{b}")
            nc.gpsimd.dma_start(taurow[32:34, :], tau_dram[b].partition_broadcast(2))
            taujp = taurow[32:34, :].rearrange("q (p j) -> q j p", j=NT)
            st2[b] = taujp

        # ---- conv phase: both batches back to back (dense PE stream) ----
        for b in range(BLOC):
            taujp = st2[b]
            xta_sb = xtas[b]
            nc.tensor.ldweights(taujp[:, 0, :], tile_position=(32, 0))
            nc.tensor.ldweights(xta_sb[:, 0:P])

            # ---- conv + pe + tau embedding, 4-tile output groups ----
            # (last group of the last batch splits in two so the drain tail
            # after the final matmul is shorter)
            groups = [(g * OG, OG) for g in range(NT // OG)]
            if b == BLOC - 1:
                groups = groups[:-1] + [(NT - OG, OG // 2), (NT - OG // 2, OG // 2)]
            for gi, (i0, glen) in enumerate(groups):
                osb = outp.tile([P, glen, D], f32, tag="osb", name=f"osb{b}_{i0}")
                for q in range(glen):
                    i = i0 + q
                    ps = psA.tile([P, D], f32, tag="psa", name=f"ps{b}_{i}")
                    nc.tensor.matmul(
                        ps,
                        lhsT=xta_sb[:, i * P : (i + 1) * P],
                        rhs=wt01_sb,
                        start=True,
                        stop=False,
                    )
                    nc.tensor.matmul(
                        ps,
                        lhsT=xta_sb[C : 2 * C, i * P + 1 : (i + 1) * P + 1],
                        rhs=wt2_sb[C : 2 * C, :],
                        start=False,
                        stop=False,
                    )
                    nc.tensor.matmul(
                        ps,
                        lhsT=taujp[:, i, :],
                        rhs=tcwhl_sb[32:34, :],
                        start=False,
                        stop=True,
                        tile_position=(32, 0),
                    )
                    # psum -> sbuf with the positional-embedding add fused in
                    nc.vector.tensor_tensor(
                        osb[:, q, :], ps, pe16_sb[:, i, :], op=Alu.add
                    )
                dst = out[b, i0 * P : (i0 + glen) * P, :].rearrange(
                    "(q p) d -> p q d", p=P
                )
                if (b * (NT // OG) + gi) % 2 == 0:
                    nc.sync.dma_start(dst, osb)
                else:
                    nc.scalar.dma_start(dst, osb)


def build_bass():
    import concourse.tile as tile
    from concourse import bacc, mybir

    f32 = mybir.dt.float32
    f16 = mybir.dt.float16

    nc = bacc.Bacc(
        "TRN2",
        target_bir_lowering=False,
        debug=False,
        enable_asserts=False,
        num_devices=NCORES,
    )
    aps = {}
    aps["xin"] = nc.dram_tensor("xin", (BLOC, S, C), f32, kind="ExternalInput").ap()
    aps["xta"] = nc.dram_tensor(
        "xta", (BLOC, P, S + 2), f16, kind="ExternalInput"
    ).ap()
    aps["pe16"] = nc.dram_tensor("pe16", (S, D), f16, kind="ExternalInput").ap()
    aps["wt01"] = nc.dram_tensor("wt01", (P, D), f16, kind="ExternalInput").ap()
    aps["wt2"] = nc.dram_tensor("wt2", (C, D), f16, kind="ExternalInput").ap()
    aps["tcwhl"] = nc.dram_tensor("tcwhl", (1, 2, D), f16, kind="ExternalInput").ap()
    aps["identh"] = nc.dram_tensor("identh", (P, P), f16, kind="ExternalInput").ap()
    aps["identf"] = nc.dram_tensor("identf", (P, P), f32, kind="ExternalInput").ap()
    aps["onesrow"] = nc.dram_tensor("onesrow", (1, P), f32, kind="ExternalInput").ap()
    aps["mask1"] = nc.dram_tensor(
        "mask1", (P, NT, NT), f16, kind="ExternalInput"
    ).ap()
    aps["kcol"] = nc.dram_tensor("kcol", (32, 1), f32, kind="ExternalInput").ap()
    aps["dval"] = nc.dram_tensor("dval", (P, NT, P), f16, kind="ExternalInput").ap()
    aps["sval"] = nc.dram_tensor("sval", (P, P), f16, kind="ExternalInput").ap()
    aps["t1c"] = nc.dram_tensor("t1c", (P, NT), f32, kind="ExternalInput").ap()
    aps["lhsTd_t"] = nc.dram_tensor("lhsTd_t", (6, S), f16, kind="ExternalInput").ap()
    aps["rhsd_t"] = nc.dram_tensor("rhsd_t", (6, S), f16, kind="ExternalInput").ap()
    aps["nbt_t"] = nc.dram_tensor("nbt_t", (96, P), f16, kind="ExternalInput").ap()
    aps["out"] = nc.dram_tensor("out", (BLOC, S, D), f32, kind="ExternalOutput").ap()
    aps["js_dram"] = nc.dram_tensor("js_scr", (BLOC, S), f16, kind="Internal").ap()
    aps["spT_d"] = nc.dram_tensor("spT_d", (BLOC, 96, P), f16, kind="Internal").ap()
    aps["tau_dram"] = nc.dram_tensor(
        "tau_scratch", (BLOC, S), f32, kind="Internal"
    ).ap()

    with tile.TileContext(nc) as tc:
        _emit(tc, aps)
    nc.compile()
    return nc


def make_consts():
    position = np.arange(S, dtype=np.float32)[:, None]
    div_term = np.exp(
        np.arange(0, D, 2, dtype=np.float32) * np.float32(-math.log(10000.0) / D)
    ).astype(np.float32)
    ang = (position * div_term).astype(np.float32)
    pe = np.zeros((S, D), dtype=np.float32)
    pe[:, 0::2] = np.sin(ang)
    pe[:, 1::2] = np.cos(ang)

    pp = np.arange(P)
    ii = np.arange(NT)
    uu = np.arange(P)
    jj = np.arange(NT)
    # mask1[p, i, j] = (j < i) * (j + 1)
    mask1 = ((jj[None, :] < ii[:, None]) * (jj[None, :] + 1.0))[None].repeat(P, 0)
    # dval[p, i, u] = (u < p) ? i*128 + u + 1 : 0
    dval = (uu[None, None, :] < pp[:, None, None]) * (
        ii[None, :, None] * P + uu[None, None, :] + 1.0
    )
    sval = np.broadcast_to(uu[None, :] + 1.0, (P, P)).copy()
    consts = {
        "identh": np.eye(P, dtype=np.float16),
        "identf": np.eye(P, dtype=np.float32),
        "onesrow": np.ones((1, P), dtype=np.float32),
        "mask1": mask1.astype(np.float16),
        "kcol": np.array([[k] for k in list(range(1, 17)) + [0] + [99] * 15], dtype=np.float32),
        "dval": dval.astype(np.float16),
        "sval": sval.astype(np.float16),
        "t1c": (ii[None, :] * P + pp[:, None] + 1.0).astype(np.float32),
        "lhsTd_t": _lhsTd_t(),
        "rhsd_t": _rhsd_t(),
        "nbt_t": _nbt_t(),
    }
    return pe, consts


def _lhsTd_t():
    t = np.zeros((6, S), dtype=np.float16)
    t[0::2] = 1.0
    return t


def _rhsd_t():
    t = np.zeros((6, S), dtype=np.float16)
    t[1::2] = 1.0
    return t


def _nbt_t():
    # rows g*32+0..15 = -v tables (overwritten); g*32+16 = "none" row
    # (-BIG in hi group, 0 in mid/lo); g*32+17 = ones (pairs with q in lhsT)
    t = np.zeros((96, P), dtype=np.float16)
    t[16] = -BIG
    t[17::32] = 1.0
    return t


def make_shared_inputs(conv_w, tc_w, tc_b):
    pe, consts = make_consts()
    pe_b = (pe + np.asarray(tc_b, np.float32)[None, :]).astype(np.float32)
    wt = np.transpose(np.asarray(conv_w, np.float32), (2, 1, 0))  # (k, c, d)
    wt01 = np.concatenate([wt[0], wt[1]], axis=0).astype(np.float16)
    wt2 = wt[2].astype(np.float16)
    w = np.asarray(tc_w, np.float32)[:, 0]
    w_hi = w.astype(np.float16)
    w_lo = (w - w_hi.astype(np.float32)).astype(np.float16)
    tcwhl = np.stack([w_hi, w_lo], axis=0)[None]
    return {
        "pe16": pe_b.astype(np.float16),
        "wt01": np.ascontiguousarray(wt01),
        "wt2": np.ascontiguousarray(wt2),
        "tcwhl": np.ascontiguousarray(tcwhl),
        **{k: np.ascontiguousarray(v) for k, v in consts.items()},
    }


def make_xta(x16):
    bl = x16.shape[0]
    xt = np.transpose(x16, (0, 2, 1))  # (bl, C, S)
    xta = np.zeros((bl, P, S + 2), dtype=np.float16)
    xta[:, 0:C, 1 : S + 1] = xt
    xta[:, 0:C, 0] = xt[:, :, S - 1]
    xta[:, C : 2 * C, 0:S] = xt
    xta[:, C : 2 * C, S] = xt[:, :, 0]
    return xta


_BUILD_CACHE = {}


def _install_ntff_hook():
    import sys as _sys
    import types

    if "antenv.axon_hooks" in _sys.modules:
        return
    try:
        from trn_agent_boot.trn_boot import _ntff_profile_via_ctypes

        hook = _ntff_profile_via_ctypes("/opt/axon/libaxon_pjrt.so")
        m = types.ModuleType("antenv.axon_hooks")
        m.get_axon_ntff_profile_hook = lambda: hook
        _sys.modules["antenv.axon_hooks"] = m
    except Exception as e:
        print("[kernel] ntff hook install failed:", e)


def kernel(x, conv_w, tc_w, tc_b):
    x = np.ascontiguousarray(np.asarray(x, dtype=np.float32))
    conv_w = np.asarray(conv_w, dtype=np.float32)
    tc_w = np.asarray(tc_w, dtype=np.float32)
    tc_b = np.asarray(tc_b, dtype=np.float32)
    assert x.shape == (B, S, C), x.shape

    from concourse.bass_utils import run_bass_kernel_spmd

    if "nc" not in _BUILD_CACHE:
        _BUILD_CACHE["nc"] = build_bass()
    nc = _BUILD_CACHE["nc"]

    shared = make_shared_inputs(conv_w, tc_w, tc_b)
    x16 = x.astype(np.float16)
    in_maps = []
    for c in range(NCORES):
        m = dict(shared)
        m["xin"] = np.ascontiguousarray(x[c * BLOC : (c + 1) * BLOC])
        m["xta"] = make_xta(x16[c * BLOC : (c + 1) * BLOC])
        in_maps.append(m)

    trace = bool(int(os.environ.get("KERNEL_TRACE", "0")))
    if trace:
        _install_ntff_hook()
    res = run_bass_kernel_spmd(
        nc, in_maps, core_ids=list(range(NCORES)), trace=trace, trace_cores=[0]
    )
    if trace and res.exec_time_ns is not None:
        print(
            f"[kernel] HW exec time: {res.exec_time_ns} ns "
            f"(mean {res.mean_exec_time_ns} ns)"
        )
        kernel.last_exec_time_ns = res.exec_time_ns
        kernel.last_trace = res.instructions_and_trace
    out = np.concatenate([r["out"] for r in res.results], axis=0)
    return out


if __name__ == "__main__":
    build_bass()
    print("build ok")


# revision 19
# speedup vs baseline: 1.2213x; 1.2213x over previous
"""Trainium2 Bass kernel for nn_DataEmbedding_v2 — v3.

Same contract as before. v3 replaces the O(S^2) DVE compare/max tc-plane with a
two-level block algorithm:
  - norms per 128-token block -> block minima (exact f32 via PE transpose)
  - level 1: for each t, last block j* with blockmin_j < thresh[t] (tiny DVE)
  - level 2: gather block j*'s norms via a one-hot PE matmul that also forms
    d = thresh[t] - norms[s] exactly (f16 hi/mid/lo 3-way splits, interleaved
    accumulation order so all partial sums stay cancellation-exact)
  - diag block (s in t's own block) via the same d-matmul with static rhs rows
  - ACT Sign(d) -> +-1, DVE multiply by (s+1) value masks, max-reduce; negative
    candidates lose automatically, "none" stays <= 0.
All compares bit-match the reference's f32 rounding.
"""

import math
import os
import sys

sys.path.insert(0, "/opt/trn_rl_repo")

import numpy as np

B, S, C, D = 16, 2048, 64, 512
NCORES = 8
BLOC = B // NCORES
P = 128
NT = S // P
ETA = 0.3
EPS = 1e-08
OG = 4  # output tiles per DMA group
BIG = 60000.0


def _emit(tc, aps):
    from contextlib import ExitStack

    from concourse import mybir

    f32 = mybir.dt.float32
    f16 = mybir.dt.float16
    Alu = mybir.AluOpType
    Ax = mybir.AxisListType
    Af = mybir.ActivationFunctionType

    nc = tc.nc
    xin, pe16, wt01, wt2 = aps["xin"], aps["pe16"], aps["wt01"], aps["wt2"]
    tcwhl, identh, identf = aps["tcwhl"], aps["identh"], aps["identf"]
    onesrow, mask1, kcol = aps["onesrow"], aps["mask1"], aps["kcol"]
    dval, sval, t1c = aps["dval"], aps["sval"], aps["t1c"]
    out, xta = aps["out"], aps["xta"]
    lhsTd_t, rhsd_t, nbt_t = aps["lhsTd_t"], aps["rhsd_t"], aps["nbt_t"]
    js_dram, tau_dram = aps["js_dram"], aps["tau_dram"]
    spT_d = aps["spT_d"]

    with ExitStack() as ctx:
        singles = ctx.enter_context(tc.tile_pool(name="singles", bufs=1))
        xpool = ctx.enter_context(tc.tile_pool(name="xpool", bufs=2))
        small = ctx.enter_context(tc.tile_pool(name="small", bufs=2))
        selp = ctx.enter_context(tc.tile_pool(name="selp", bufs=2))
        xtp = ctx.enter_context(tc.tile_pool(name="xtp", bufs=2))
        sgp = ctx.enter_context(tc.tile_pool(name="sgp", bufs=2))
        outp = ctx.enter_context(tc.tile_pool(name="outp", bufs=3))
        psA = ctx.enter_context(tc.tile_pool(name="psA", bufs=3, space="PSUM"))
        psT = ctx.enter_context(tc.tile_pool(name="psT", bufs=1, space="PSUM"))
        psD = ctx.enter_context(tc.tile_pool(name="psD", bufs=2, space="PSUM"))

        # ---- x loads (scalar/gpsimd rings; sync ring starts on consts) ----
        xins = {}
        xtas = {}
        for b in range(BLOC):
            xin_sb = xpool.tile([P, NT, C], f32, tag="xin", name=f"xin_sb{b}")
            nc.scalar.dma_start(xin_sb, xin[b].rearrange("(j p) c -> p j c", p=P))
            xins[b] = xin_sb
            xta_sb = xtp.tile([P, S + 2], f16, tag="xta", name=f"xta{b}")
            nc.gpsimd.dma_start(xta_sb, xta[b])
            xtas[b] = xta_sb

        # ---- constants ----
        identh_sb = singles.tile([P, P], f16)
        nc.sync.dma_start(identh_sb, identh)
        identf_sb = singles.tile([P, P], f32)
        nc.sync.dma_start(identf_sb, identf)
        onesrow_sb = singles.tile([1, P], f32)
        nc.sync.dma_start(onesrow_sb, onesrow)
        mask1_sb = singles.tile([P, NT, NT], f16)
        nc.sync.dma_start(mask1_sb, mask1)
        kcol_sb = singles.tile([32, 1], f32)
        nc.sync.dma_start(kcol_sb, kcol)
        dval_sb = singles.tile([P, NT, P], f16)
        nc.sync.dma_start(dval_sb, dval)
        sval_sb = singles.tile([P, P], f16)
        nc.sync.dma_start(sval_sb, sval)
        t1_sb = singles.tile([P, NT], f32)
        nc.sync.dma_start(t1_sb, t1c)
        wt01_sb = singles.tile([P, D], f16)
        nc.sync.dma_start(wt01_sb, wt01)
        wt2_sb = singles.tile([P, D], f16)
        nc.sync.dma_start(wt2_sb[C : 2 * C, :], wt2)
        tcwhl_sb = singles.tile([34, D], f16)
        nc.sync.dma_start(tcwhl_sb[32:34, :], tcwhl[0])
        pe16_sb = singles.tile([P, NT, D], f16)
        nc.sync.dma_start(pe16_sb, pe16.rearrange("(i p) d -> p i d", p=P))

        # ---- PE priming: absorb each const-DMA dependency once ----
        prime_h = psA.tile([P, P], f16, tag="psa")
        nc.tensor.transpose(prime_h, identh_sb, identh_sb)
        prime_f = psA.tile([P, P], f32, tag="psa")
        nc.tensor.transpose(prime_f, identf_sb, identf_sb)
        prime_w2 = psA.tile([P, D], f32, tag="psa")
        nc.tensor.matmul(
            prime_w2[C : 2 * C, :],
            lhsT=wt2_sb[C : 2 * C, 0:C],
            rhs=wt2_sb[C : 2 * C, :],
            start=True,
            stop=True,
        )
        prime_tc = psA.tile([P, D], f32, tag="psa")
        nc.tensor.matmul(
            prime_tc,
            lhsT=tcwhl_sb[32:34, 0:P],
            rhs=tcwhl_sb[32:34, :],
            start=True,
            stop=True,
            tile_position=(32, 0),
        )
        prime_w01 = psA.tile([P, D], f32, tag="psa")
        nc.tensor.matmul(
            prime_w01, lhsT=wt01_sb[:, 0:P], rhs=wt01_sb, start=True, stop=True
        )
        prime_on = psA.tile([P, 16], f32, tag="psa")
        nc.tensor.matmul(
            prime_on, lhsT=onesrow_sb, rhs=onesrow_sb[:, 0:16], start=True, stop=True
        )
        dumps = singles.tile([P, 7], f32)
        nc.scalar.copy(dumps[:, 0:1], prime_h[:, 0:1])
        nc.scalar.copy(dumps[:, 1:2], prime_f[:, 0:1])
        nc.scalar.copy(dumps[C : 2 * C, 2:3], prime_w2[C : 2 * C, 0:1])
        nc.scalar.copy(dumps[:, 3:4], prime_tc[:, 0:1])
        nc.scalar.copy(dumps[:, 4:5], prime_w01[:, 0:1])
        nc.scalar.copy(dumps[:, 6:7], prime_on[:, 0:1])

        # ---- Phase A per batch: norms, splits, block minima, level 1 ----
        st = {}
        for b in range(BLOC):
            xin_sb = xins[b]
            r8 = small.tile([P, NT, 8], f32, tag="r8", name=f"r8_{b}")
            nc.vector.tensor_reduce(
                r8,
                xin_sb.rearrange("p j (a b) -> p j a b", b=8),
                axis=Ax.X,
                op=Alu.add,
                apply_absolute_value=True,
            )
            normc = small.tile([P, NT], f32, tag="normc", name=f"normc{b}")
            nc.vector.tensor_reduce(normc, r8, axis=Ax.X, op=Alu.add)
            # negthc[:,0,:] = -norms, negthc[:,1,:] = thresh = (norms+EPS)*0.7
            negthc = small.tile([P, 2, NT], f32, tag="negthc", name=f"negthc{b}")
            nc.vector.tensor_scalar(negthc[:, 0, :], normc, -1.0, None, op0=Alu.mult)
            nc.vector.tensor_scalar(
                negthc[:, 1, :],
                normc,
                float(EPS),
                float(1.0 - ETA),
                op0=Alu.add,
                op1=Alu.mult,
            )
            # f16 3-way splits of (-norms, thresh): exact hi/mid/lo
            # layout [P, {n,q}, k, NT] so the DMA slices below are contiguous
            spl = small.tile([P, 2, 3, NT], f16, tag="spl", name=f"spl{b}")
            nc.vector.tensor_copy(spl[:, :, 0, :], negthc)
            rem1 = small.tile([P, 2, NT], f32, tag="rem1", name=f"rem1_{b}")
            nc.vector.tensor_tensor(rem1, negthc, spl[:, :, 0, :], op=Alu.subtract)
            nc.vector.tensor_copy(spl[:, :, 1, :], rem1)
            rem2 = small.tile([P, 2, NT], f32, tag="rem2", name=f"rem2_{b}")
            nc.vector.tensor_tensor(rem2, rem1, spl[:, :, 1, :], op=Alu.subtract)
            nc.vector.tensor_copy(spl[:, :, 2, :], rem2)
            # transpose all six split vectors at once: spT[(nq)*48 + k*16 + i, m]
            # = split[(nq), k][t = i*128 + m]  (rows 0:48 = -n, 48:96 = q)
            ps96 = psT.tile([96, P], f16, tag="pst", name=f"ps96_{b}")
            nc.tensor.transpose(ps96, spl.rearrange("p a k j -> p (a k j)"), identh_sb)
            spT = small.tile([96, P], f16, tag="spt", name=f"spT{b}")
            nc.scalar.copy(spT, ps96)
            nc.scalar.dma_start(spT_d[b], spT)
            # block minima of norms  (= -max of -norms), exact f32
            psnT = psT.tile([NT, P], f32, tag="pst", name=f"psnT{b}")
            nc.tensor.transpose(psnT, negthc[:, 0, :], identf_sb)
            bneg = small.tile([NT, 1], f32, tag="bneg", name=f"bneg{b}")
            nc.vector.tensor_reduce(bneg, psnT, axis=Ax.X, op=Alu.max)
            psb1 = psT.tile([1, NT], f32, tag="pst", name=f"psb1_{b}")
            nc.tensor.transpose(psb1, bneg, identf_sb[0:NT, 0:NT])
            bnegrow = small.tile([1, NT], f32, tag="bnegrow", name=f"bnegrow{b}")
            nc.scalar.copy(bnegrow, psb1)
            psbb = psT.tile([P, NT], f32, tag="pst", name=f"psbb{b}")
            nc.tensor.matmul(
                psbb, lhsT=onesrow_sb, rhs=bnegrow, start=True, stop=True
            )
            bnegbc = small.tile([P, NT], f32, tag="bnegbc", name=f"bnegbc{b}")
            nc.scalar.copy(bnegbc, psbb)
            # level 1: jp1 = 1 + last block j<i with bmin_j < thresh  (0 = none)
            l1a = small.tile([P, NT, NT], f32, tag="l1a", name=f"l1a{b}")
            nc.vector.tensor_tensor(
                l1a,
                negthc[:, 1, :].unsqueeze(2).to_broadcast([P, NT, NT]),
                bnegbc.unsqueeze(1).to_broadcast([P, NT, NT]),
                op=Alu.add,
            )
            l1b = small.tile([P, NT, NT], f16, tag="l1b", name=f"l1b{b}")
            nc.vector.tensor_scalar(l1b, l1a, 0.0, None, op0=Alu.is_gt)
            l1c = small.tile([P, NT, NT], f16, tag="l1c", name=f"l1c{b}")
            nc.vector.tensor_tensor(l1c, l1b, mask1_sb, op=Alu.mult)
            jp1 = small.tile([P, NT], f16, tag="jp1", name=f"jp1_{b}")
            nc.vector.tensor_reduce(jp1, l1c, axis=Ax.X, op=Alu.max)
            psj = psT.tile([NT, P], f16, tag="pst", name=f"psj{b}")
            nc.tensor.transpose(psj, jp1, identh_sb)
            jpT = small.tile([NT, P], f16, tag="jpt", name=f"jpT{b}")
            nc.scalar.copy(jpT, psj)
            nc.sync.dma_start(js_dram[b].rearrange("(i p) -> i p", p=P), jpT)
            st[b] = (negthc, spl, jp1, spT)

        # ---- Phase B per batch: select matmuls, sign, reduce, tau, conv ----
        for b in range(BLOC):
            negthc, spl, jp1, spT = st[b]
            xta_sb = xtas[b]
            # one-hot lhsT [96, S], 32-aligned groups (hi/mid/lo):
            # rows g*32+0..16 = one-hot (17), g*32+17 = q_g, rest zeroed
            ohx = selp.tile([96, S], f16, tag="ohx", name=f"ohx{b}")
            ohsrc = selp.tile([32, S], f16, tag="ohsrc", name=f"ohsrc{b}")
            nc.gpsimd.dma_start(ohsrc, js_dram[b].partition_broadcast(32))
            nc.vector.tensor_scalar(
                ohx[0:32, :], ohsrc, kcol_sb, None, op0=Alu.is_equal
            )
            nc.vector.tensor_scalar(
                ohx[32:64, :], ohsrc, kcol_sb, None, op0=Alu.is_equal
            )
            nc.vector.tensor_scalar(
                ohx[64:96, :], ohsrc, kcol_sb, None, op0=Alu.is_equal
            )
            nc.scalar.dma_start(
                ohx[17:82:32, :], spT_d[b, 48:96].rearrange("(k i) m -> k (i m)", k=3)
            )
            # diag lhsT [6, S]: ones rows 0/2/4, q rows 1/3/5
            lhsTd = selp.tile([6, S], f16, tag="lhstd", name=f"lhsTd{b}")
            nc.gpsimd.dma_start(lhsTd, lhsTd_t)
            nc.scalar.dma_start(
                lhsTd[1:6:2, :], spT_d[b, 48:96].rearrange("(k i) m -> k (i m)", k=3)
            )
            # diag rhs [6, S]: -n rows 0/2/4, ones rows 1/3/5
            rhsd = selp.tile([6, S], f16, tag="rhsd", name=f"rhsd{b}")
            nc.gpsimd.dma_start(rhsd, rhsd_t)
            nc.scalar.dma_start(
                rhsd[0:6:2, :], spT_d[b, 0:48].rearrange("(k i) m -> k (i m)", k=3)
            )
            # one-hot rhs tables [54, P]: -n blocks hi/mid/lo + specials
            nbt = selp.tile([96, P], f16, tag="nbt", name=f"nbt{b}")
            nc.gpsimd.dma_start(nbt, nbt_t)
            for g in range(3):
                nc.sync.dma_start(
                    nbt[g * 32 : g * 32 + 16, :], spT_d[b, g * 16 : (g + 1) * 16]
                )

            # absorb multi-writer waits before the select matmuls
            nc.tensor.ldweights(lhsTd[:, 0:P])
            nc.tensor.ldweights(rhsd[:, 0:P])
            nc.tensor.ldweights(ohx[:, 0:P])
            nc.tensor.ldweights(nbt[:, 0:P])

            rdiag = small.tile([P, NT], f16, tag="rdiag", name=f"rdiag{b}")
            rsel = small.tile([P, NT], f16, tag="rsel", name=f"rsel{b}")
            for h in range(4):
                psd = psD.tile([P, 4, 2, P], f32, tag="psd", name=f"psd{b}_{h}")
                for ii in range(4):
                    i = h * 4 + ii
                    sl = slice(i * P, (i + 1) * P)
                    nc.tensor.matmul(
                        psd[:, ii, 0, :],
                        lhsT=lhsTd[:, sl],
                        rhs=rhsd[:, sl],
                        start=True,
                        stop=True,
                    )
                    nc.tensor.matmul(
                        psd[:, ii, 1, :],
                        lhsT=ohx[:, sl],
                        rhs=nbt,
                        start=True,
                        stop=True,
                    )
                sgn = sgp.tile([P, 4, 2, P], f16, tag="sgn", name=f"sgn{b}_{h}")
                nc.scalar.activation(sgn, psd, Af.Sign)
                dm = sgp.tile([P, 4, P], f16, tag="dm", name=f"dm{b}_{h}")
                nc.vector.tensor_tensor(
                    dm, sgn[:, :, 0, :], dval_sb[:, h * 4 : (h + 1) * 4, :],
                    op=Alu.mult,
                )
                sm = sgp.tile([P, 4, P], f16, tag="sm", name=f"sm{b}_{h}")
                nc.vector.tensor_tensor(
                    sm,
                    sgn[:, :, 1, :],
                    sval_sb.unsqueeze(1).to_broadcast([P, 4, P]),
                    op=Alu.mult,
                )
                nc.vector.tensor_reduce(
                    rdiag[:, h * 4 : (h + 1) * 4], dm, axis=Ax.X, op=Alu.max
                )
                nc.vector.tensor_reduce(
                    rsel[:, h * 4 : (h + 1) * 4], sm, axis=Ax.X, op=Alu.max
                )

            # ans = max(diag, (jp1-1)*128 + rsel);  tau = ans>0 ? t+1-ans : 0
            cand2 = small.tile([P, NT], f16, tag="cand2", name=f"cand2_{b}")
            nc.vector.tensor_scalar(
                cand2, jp1, 128.0, -128.0, op0=Alu.mult, op1=Alu.add
            )
            cand2b = small.tile([P, NT], f16, tag="cand2b", name=f"cand2b{b}")
            nc.vector.tensor_tensor(cand2b, cand2, rsel, op=Alu.add)
            ans = small.tile([P, NT], f16, tag="ans", name=f"ans{b}")
            nc.vector.tensor_tensor(ans, cand2b, rdiag, op=Alu.max)
            m01 = small.tile([P, NT], f16, tag="m01", name=f"m01_{b}")
            nc.vector.tensor_scalar(m01, ans, 0.0, None, op0=Alu.is_gt)
            td = small.tile([P, NT], f32, tag="td", name=f"td{b}")
            nc.vector.tensor_tensor(td, t1_sb, ans, op=Alu.subtract)
            tauc = small.tile([P, NT], f32, tag="tauc", name=f"tauc{b}")
            nc.vector.tensor_tensor(tauc, td, m01, op=Alu.mult)
            nc.sync.dma_start(tau_dram[b].rearrange("(p j) -> p j", p=P), tauc)
            taurow = small.tile([34, S], f16, tag="taurow", name=f"taurow{b}")
            nc.gpsimd.dma_start(taurow[32:34, :], tau_dram[b].partition_broadcast(2))
            taujp = taurow[32:34, :].rearrange("q (p j) -> q j p", j=NT)
            nc.tensor.ldweights(taujp[:, 0, :], tile_position=(32, 0))
            nc.tensor.ldweights(xta_sb[:, 0:P])

            # ---- conv + pe + tau embedding, 4-tile output groups ----
            # (last group of the last batch splits in two so the drain tail
            # after the final matmul is shorter)
            groups = [(g * OG, OG) for g in range(NT // OG)]
            if b == BLOC - 1:
                groups = groups[:-1] + [(NT - OG, OG // 2), (NT - OG // 2, OG // 2)]
            for gi, (i0, glen) in enumerate(groups):
                osb = outp.tile([P, glen, D], f32, tag="osb", name=f"osb{b}_{i0}")
                for q in range(glen):
                    i = i0 + q
                    ps = psA.tile([P, D], f32, tag="psa", name=f"ps{b}_{i}")
                    nc.tensor.matmul(
                        ps,
                        lhsT=xta_sb[:, i * P : (i + 1) * P],
                        rhs=wt01_sb,
                        start=True,
                        stop=False,
                    )
                    nc.tensor.matmul(
                        ps,
                        lhsT=xta_sb[C : 2 * C, i * P + 1 : (i + 1) * P + 1],
                        rhs=wt2_sb[C : 2 * C, :],
                        start=False,
                        stop=False,
                    )
                    nc.tensor.matmul(
                        ps,
                        lhsT=taujp[:, i, :],
                        rhs=tcwhl_sb[32:34, :],
                        start=False,
                        stop=True,
                        tile_position=(32, 0),
                    )
                    # psum -> sbuf with the positional-embedding add fused in
                    nc.vector.tensor_tensor(
                        osb[:, q, :], ps, pe16_sb[:, i, :], op=Alu.add
                    )
                dst = out[b, i0 * P : (i0 + glen) * P, :].rearrange(
                    "(q p) d -> p q d", p=P
                )
                if (b * (NT // OG) + gi) % 2 == 0:
                    nc.sync.dma_start(dst, osb)
                else:
                    nc.scalar.dma_start(dst, osb)


def build_bass():
    import concourse.tile as tile
    from concourse import bacc, mybir

    f32 = mybir.dt.float32
    f16 = mybir.dt.float16

    nc = bacc.Bacc(
        "TRN2",
        target_bir_lowering=False,
        debug=False,
        enable_asserts=False,
        num_devices=NCORES,
    )
    aps = {}
    aps["xin"] = nc.dram_tensor("xin", (BLOC, S, C), f32, kind="ExternalInput").ap()
    aps["xta"] = nc.dram_tensor(
        "xta", (BLOC, P, S + 2), f16, kind="ExternalInput"
    ).ap()
    aps["pe16"] = nc.dram_tensor("pe16", (S, D), f16, kind="ExternalInput").ap()
    aps["wt01"] = nc.dram_tensor("wt01", (P, D), f16, kind="ExternalInput").ap()
    aps["wt2"] = nc.dram_tensor("wt2", (C, D), f16, kind="ExternalInput").ap()
    aps["tcwhl"] = nc.dram_tensor("tcwhl", (1, 2, D), f16, kind="ExternalInput").ap()
    aps["identh"] = nc.dram_tensor("identh", (P, P), f16, kind="ExternalInput").ap()
    aps["identf"] = nc.dram_tensor("identf", (P, P), f32, kind="ExternalInput").ap()
    aps["onesrow"] = nc.dram_tensor("onesrow", (1, P), f32, kind="ExternalInput").ap()
    aps["mask1"] = nc.dram_tensor(
        "mask1", (P, NT, NT), f16, kind="ExternalInput"
    ).ap()
    aps["kcol"] = nc.dram_tensor("kcol", (32, 1), f32, kind="ExternalInput").ap()
    aps["dval"] = nc.dram_tensor("dval", (P, NT, P), f16, kind="ExternalInput").ap()
    aps["sval"] = nc.dram_tensor("sval", (P, P), f16, kind="ExternalInput").ap()
    aps["t1c"] = nc.dram_tensor("t1c", (P, NT), f32, kind="ExternalInput").ap()
    aps["lhsTd_t"] = nc.dram_tensor("lhsTd_t", (6, S), f16, kind="ExternalInput").ap()
    aps["rhsd_t"] = nc.dram_tensor("rhsd_t", (6, S), f16, kind="ExternalInput").ap()
    aps["nbt_t"] = nc.dram_tensor("nbt_t", (96, P), f16, kind="ExternalInput").ap()
    aps["out"] = nc.dram_tensor("out", (BLOC, S, D), f32, kind="ExternalOutput").ap()
    aps["js_dram"] = nc.dram_tensor("js_scr", (BLOC, S), f16, kind="Internal").ap()
    aps["spT_d"] = nc.dram_tensor("spT_d", (BLOC, 96, P), f16, kind="Internal").ap()
    aps["tau_dram"] = nc.dram_tensor(
        "tau_scratch", (BLOC, S), f32, kind="Internal"
    ).ap()

    with tile.TileContext(nc) as tc:
        _emit(tc, aps)
    nc.compile()
    return nc


def make_consts():
    position = np.arange(S, dtype=np.float32)[:, None]
    div_term = np.exp(
        np.arange(0, D, 2, dtype=np.float32) * np.float32(-math.log(10000.0) / D)
    ).astype(np.float32)
    ang = (position * div_term).astype(np.float32)
    pe = np.zeros((S, D), dtype=np.float32)
    pe[:, 0::2] = np.sin(ang)
    pe[:, 1::2] = np.cos(ang)

    pp = np.arange(P)
    ii = np.arange(NT)
    uu = np.arange(P)
    jj = np.arange(NT)
    # mask1[p, i, j] = (j < i) * (j + 1)
    mask1 = ((jj[None, :] < ii[:, None]) * (jj[None, :] + 1.0))[None].repeat(P, 0)
    # dval[p, i, u] = (u < p) ? i*128 + u + 1 : 0
    dval = (uu[None, None, :] < pp[:, None, None]) * (
        ii[None, :, None] * P + uu[None, None, :] + 1.0
    )
    sval = np.broadcast_to(uu[None, :] + 1.0, (P, P)).copy()
    consts = {
        "identh": np.eye(P, dtype=np.float16),
        "identf": np.eye(P, dtype=np.float32),
        "onesrow": np.ones((1, P), dtype=np.float32),
        "mask1": mask1.astype(np.float16),
        "kcol": np.array([[k] for k in list(range(1, 17)) + [0] + [99] * 15], dtype=np.float32),
        "dval": dval.astype(np.float16),
        "sval": sval.astype(np.float16),
        "t1c": (ii[None, :] * P + pp[:, None] + 1.0).astype(np.float32),
        "lhsTd_t": _lhsTd_t(),
        "rhsd_t": _rhsd_t(),
        "nbt_t": _nbt_t(),
    }
    return pe, consts


def _lhsTd_t():
    t = np.zeros((6, S), dtype=np.float16)
    t[0::2] = 1.0
    return t


def _rhsd_t():
    t = np.zeros((6, S), dtype=np.float16)
    t[1::2] = 1.0
    return t


def _nbt_t():
    # rows g*32+0..15 = -v tables (overwritten); g*32+16 = "none" row
    # (-BIG in hi group, 0 in mid/lo); g*32+17 = ones (pairs with q in lhsT)
    t = np.zeros((96, P), dtype=np.float16)
    t[16] = -BIG
    t[17::32] = 1.0
    return t


def make_shared_inputs(conv_w, tc_w, tc_b):
    pe, consts = make_consts()
    pe_b = (pe + np.asarray(tc_b, np.float32)[None, :]).astype(np.float32)
    wt = np.transpose(np.asarray(conv_w, np.float32), (2, 1, 0))  # (k, c, d)
    wt01 = np.concatenate([wt[0], wt[1]], axis=0).astype(np.float16)
    wt2 = wt[2].astype(np.float16)
    w = np.asarray(tc_w, np.float32)[:, 0]
    w_hi = w.astype(np.float16)
    w_lo = (w - w_hi.astype(np.float32)).astype(np.float16)
    tcwhl = np.stack([w_hi, w_lo], axis=0)[None]
    return {
        "pe16": pe_b.astype(np.float16),
        "wt01": np.ascontiguousarray(wt01),
        "wt2": np.ascontiguousarray(wt2),
        "tcwhl": np.ascontiguousarray(tcwhl),
        **{k: np.ascontiguousarray(v) for k, v in consts.items()},
    }


def make_xta(x16):
    bl = x16.shape[0]
    xt = np.transpose(x16, (0, 2, 1))  # (bl, C, S)
    xta = np.zeros((bl, P, S + 2), dtype=np.float16)
    xta[:, 0:C, 1 : S + 1] = xt
    xta[:, 0:C, 0] = xt[:, :, S - 1]
    xta[:, C : 2 * C, 0:S] = xt
    xta[:, C : 2 * C, S] = xt[:, :, 0]
    return xta


_BUILD_CACHE = {}


def _install_ntff_hook():
    import sys as _sys
    import types

    if "antenv.axon_hooks" in _sys.modules:
        return
    try:
        from trn_agent_boot.trn_boot import _ntff_profile_via_ctypes

        hook = _ntff_profile_via_ctypes("/opt/axon/libaxon_pjrt.so")
        m = types.ModuleType("antenv.axon_hooks")
        m.get_axon_ntff_profile_hook = lambda: hook
        _sys.modules["antenv.axon_hooks"] = m
    except Exception as e:
        print("[kernel] ntff hook install failed:", e)


def kernel(x, conv_w, tc_w, tc_b):
    x = np.ascontiguousarray(np.asarray(x, dtype=np.float32))
    conv_w = np.asarray(conv_w, dtype=np.float32)
    tc_w = np.asarray(tc_w, dtype=np.float32)
    tc_b = np.asarray(tc_b, dtype=np.float32)
    assert x.shape == (B, S, C), x.shape

    from concourse.bass_utils import run_bass_kernel_spmd

    if "nc" not in _BUILD_CACHE:
        _BUILD_CACHE["nc"] = build_bass()
    nc = _BUILD_CACHE["nc"]

    shared = make_shared_inputs(conv_w, tc_w, tc_b)
    x16 = x.astype(np.float16)
    in_maps = []
    for c in range(NCORES):
        m = dict(shared)
        m["xin"] = np.ascontiguousarray(x[c * BLOC : (c + 1) * BLOC])
        m["xta"] = make_xta(x16[c * BLOC : (c + 1) * BLOC])
        in_maps.append(m)

    trace = bool(int(os.environ.get("KERNEL_TRACE", "0")))
    if trace:
        _install_ntff_hook()
    res = run_bass_kernel_spmd(
        nc, in_maps, core_ids=list(range(NCORES)), trace=trace, trace_cores=[0]
    )
    if trace and res.exec_time_ns is not None:
        print(
            f"[kernel] HW exec time: {res.exec_time_ns} ns "
            f"(mean {res.mean_exec_time_ns} ns)"
        )
        kernel.last_exec_time_ns = res.exec_time_ns
        kernel.last_trace = res.instructions_and_trace
    out = np.concatenate([r["out"] for r in res.results], axis=0)
    return out


if __name__ == "__main__":
    build_bass()
    print("build ok")
